# revision 1
# baseline (speedup 1.0000x reference)
"""Trainium2 Bass kernel for nn_DenoisingModule (non-local attention block).

Reference computation (per batch element n, with C=256 channels, HW=4096):
    theta = W_t x + b_t            # queries  [C, HW]
    phi   = W_p x + b_p            # keys     [C, HW]
    g     = x                      # values   [C, HW]
    S     = theta^T phi / sqrt(C)  # [HW, HW]
    A     = softmax(S, axis=keys)
    f     = g A^T                  # [C, HW]
    out   = x + W_c f + b_c

Sharding: 8 cores; each of the N=4 batch elements is split across 2 cores
by query position (2048 queries per core). Every core holds the full key
set for its batch element, so no collectives are needed.

Per-core device program (SPMD, identical on all cores, data differs):
  - scores are computed TRANSPOSED (S^T[q, p] = phi^T theta) so that the
    softmax key-reduction lands on the PSUM partition axis, which lets the
    exp output E^T[q, p] feed the PV matmul directly (no PE transposes).
  - softmax denominators accumulate on the Vector engine (pairwise adds
    of the exp tiles), finishing with a ones-row matmul partition-reduce;
    1/Sum comes from ACT ln/exp (same activation-table set as the softmax
    exp) and is applied to f right before the output projection (the
    normalization commutes with the channel-mixing conv).
  - residual + bias are fused into the final PSUM eviction.
  - the attention loop is software-pipelined (PV trails scores/exp by one
    pair; per-group normalize/conv work is deferred into the next group).

Toolchain constraint that shapes this file: every TPB engine instruction
(and every DMA) may carry at most ONE semaphore wait, so cross-engine
fan-in is funneled through per-engine collector chains, persistent ring
tiles replace rotating tile pools, and loads/stores are merged so each
DMA is the first instruction on its hardware queue.

The host wrapper rolls x columns per-core so queries are always columns
[0, P) of the local key matrix (keeps the program identical across cores),
and pre-transposes x (and the weight matrices) since the PV matmul needs
x^T as the stationary operand.
"""

import numpy as np

import concourse.bass as bass
import concourse.mybir as mybir
from concourse import bacc
from concourse.bass_utils import run_bass_kernel_spmd
from concourse.tile import TileContext, add_dep_helper

N, C, H, W = 4, 256, 64, 64
HW = H * W
NCORES = 8
CORES_PER_N = NCORES // N
P_CORE = HW // CORES_PER_N  # queries per core

F32 = mybir.dt.float32


def build_program(P, Q, Cc=C, mm_dt=mybir.dt.float16):
    """Build the per-core Bass program.

    P: queries handled by this core (first P columns of xk)
    Q: total key positions
    mm_dt: dtype for matmul operands (float32r = relaxed single-pass fp32)
    """
    assert P % 512 == 0 and Q % 512 == 0 and Cc % 128 == 0
    CT = Cc // 128
    QT = Q // 128
    PG = P // 512
    QG = Q // 512
    scale = float(Cc) ** -0.5

    nc = bacc.Bacc("TRN2", target_bir_lowering=False)
    xk = nc.declare_dram_parameter("xk", [Cc, Q], mm_dt, isOutput=False)[:]
    xq = nc.declare_dram_parameter("xq", [Cc, P], F32, isOutput=False)[:]
    xt = nc.declare_dram_parameter("xt", [Q, Cc], mm_dt, isOutput=False)[:]
    wcat = nc.declare_dram_parameter("wcat", [3, Cc, Cc], mm_dt, isOutput=False)[:]
    bcat = nc.declare_dram_parameter("bcat", [3, Cc], F32, isOutput=False)[:]
    out = nc.declare_dram_parameter("out", [Cc, P], F32, isOutput=True)[:]

    add = mybir.AluOpType.add

    with TileContext(nc) as tc:
        with (
            tc.tile_pool(name="const", bufs=1) as const,
            tc.tile_pool(name="big", bufs=1) as big,
            tc.tile_pool(name="pss", bufs=1, space="PSUM") as pss,
            tc.tile_pool(name="psf", bufs=1, space="PSUM") as psf,
            tc.tile_pool(name="pso", bufs=1, space="PSUM") as pso,
        ):
            QH = Q // 2

            # ---- input loads: 6 DMAs, one per HWDGE proc ----
            ws_sb = const.tile([128, 3, CT, Cc], mm_dt, tag="ws")
            w_load = nc.sync.dma_start(
                out=ws_sb, in_=wcat.rearrange("w (a p) o -> p w a o", p=128)
            )
            xk_sb = big.tile([128, CT, Q], mm_dt, tag="xk")
            xk_loads = [
                nc.sync.dma_start(
                    out=xk_sb[:, :, h * QH : (h + 1) * QH],
                    in_=xk[:, h * QH : (h + 1) * QH].rearrange(
                        "(a p) q -> p a q", p=128
                    ),
                )
                for h in range(2)
            ]
            bb = const.tile([128, 3, CT], F32, tag="bb")
            b_load = nc.sync.dma_start(
                out=bb, in_=bcat.rearrange("w (a p) -> p w a", p=128)
            )
            xq_sb = big.tile([128, CT, P], F32, tag="xq")
            xq_load = nc.sync.dma_start(
                out=xq_sb, in_=xq.rearrange("(a p) q -> p a q", p=128)
            )
            xt_sb = big.tile([128, QT, Cc], mm_dt, tag="xt")
            xt_load = nc.sync.dma_start(
                out=xt_sb, in_=xt.rearrange("(a p) c -> p a c", p=128)
            )

            # persistent tiles (deliberately NOT pool-rotated: pool-slot
            # releases fan in multiple procs; rings keep wait fan-in low)
            th_sb = big.tile([128, CT, P], mm_dt, tag="th")
            ph_sb = big.tile([128, CT, Q], mm_dt, tag="ph")
            f_sb = big.tile([128, CT, P], mm_dt, tag="f")
            e_ring = big.tile([128, 4, 2, 512], mm_dt, tag="ering")
            acc = big.tile([128, 512], mybir.dt.float32r, tag="acc")
            acc2 = big.tile([128, 512], mybir.dt.float32r, tag="acc2")
            tsum = big.tile([128, 512], mybir.dt.float32r, tag="tsum")
            rc_ring = const.tile([1, PG, 512], mm_dt, tag="rcring")
            lns = const.tile([1, PG, 512], F32, tag="lns")
            bc_ring = big.tile([128, PG, 512], F32, tag="bcring")
            o_ring = big.tile([128, CT, PG, 512], F32, tag="oring")

            # ---- engine program-order chains + wait collectors ----
            last = {}

            def chain(eng, inst):
                # ordering edges disabled: Bacc legalizes multi-waits, so the
                # Tile scheduler is free to interleave within each engine
                last[eng] = inst.ins
                return inst

            ones_f = const.tile([128, 1], F32, tag="ones_f")
            chain("v", nc.vector.memset(ones_f, 1.0))
            ones_col = const.tile([1, 128], mm_dt, tag="ones_col")
            chain("v", nc.vector.tensor_copy(
                ones_col, ones_f[0:1, 0:1].to_broadcast([1, 128])))
            ones_r = const.tile([128, 1], mybir.dt.float32r, tag="ones_r")
            chain("v", nc.vector.tensor_copy(ones_r, ones_f))
            zbias = const.tile([128, 1], F32, tag="zbias")
            zb_inst = chain("v", nc.vector.memset(zbias, 0.0))

            scr_act = const.tile([1, 1], F32, tag="scr_act")
            acol = nc.scalar.activation(
                scr_act, zbias[0:1, :], mybir.ActivationFunctionType.Copy
            )
            add_dep_helper(acol.ins, zb_inst.ins, True, "act bias barrier")
            last["a"] = acol.ins

            for k, ld in enumerate([b_load, xq_load]):
                scr_k = const.tile([1, 1], F32, tag=f"scr{k}", name=f"scr{k}")
                dcol = nc.vector.memset(scr_k, 0.0)
                add_dep_helper(dcol.ins, ld.ins, True, "dve input barrier")
                chain("v", dcol)

            ps_col = pso.tile([1, 1], F32, tag="misc", name="ps_col")
            probe = bb[0:1, 0, 0:1]

            def pe_barrier(ld):
                col = nc.tensor.matmul(ps_col, lhsT=probe, rhs=probe)
                add_dep_helper(col.ins, ld.ins, True, "pe input barrier")
                chain("p", col)

            pe_barrier(w_load)
            pe_barrier(xk_loads[0])

            def mm(*args, **kwargs):
                return chain("p", nc.tensor.matmul(*args, **kwargs))

            def dve(fn, *args, **kwargs):
                return chain("v", fn(*args, **kwargs))

            def act(*args, **kwargs):
                return chain("a", nc.scalar.activation(*args, **kwargs))

            # ---- projections (paired groups share the "s" PSUM slot) ----
            def project(w_idx, dst, ngroups, bias_col, g0=0):
                for co in range(CT):
                    for g in range(g0, ngroups):
                        ps_pj = psf.tile(
                            [128, 512], F32, tag=f"f{g % 2}", name="ps_pj"
                        )
                        for ci in range(CT):
                            mm(
                                ps_pj,
                                lhsT=ws_sb[:, w_idx, ci, co * 128 : (co + 1) * 128],
                                rhs=xk_sb[:, ci, g * 512 : (g + 1) * 512],
                                start=(ci == 0),
                                stop=(ci == CT - 1),
                            )
                        act(
                            dst[:, co, g * 512 : (g + 1) * 512],
                            ps_pj,
                            mybir.ActivationFunctionType.Identity,
                            bias=bb[:, bias_col, co : co + 1],
                        )

            project(0, th_sb, PG, 0)
            project(1, ph_sb, QG // 2, 1)
            pe_barrier(xk_loads[1])
            project(1, ph_sb, QG, 1, g0=QG // 2)
            pe_barrier(xt_load)

            # ---- attention; per-group finalization is deferred into the
            # next group so the softmax-denominator chain (DVE sum tail ->
            # reduce matmul -> ACT ln/exp -> PE broadcast) overlaps PE work
            deferred = [None]
            sum_tiles = [None]

            def finalize_bc(pg):
                psl = slice(pg * 512, (pg + 1) * 512)
                ps_fs = deferred[0][1]
                ps_bc = pso.tile([128, 512], F32, tag="misc", name="ps_bc")
                mm(ps_bc, lhsT=ones_col, rhs=rc_ring[:, pg, :])
                bc_sb = bc_ring[:, pg, :]
                dve(nc.vector.tensor_copy, bc_sb, ps_bc)
                for ci in range(CT):
                    dve(
                        nc.vector.tensor_mul, f_sb[:, ci, psl], ps_fs[ci], bc_sb
                    )

            def finalize_conv(pg):
                psl = slice(pg * 512, (pg + 1) * 512)
                for co in range(CT):
                    ps_o = pso.tile([128, 512], F32, tag="o")
                    for ci in range(CT):
                        mm(
                            ps_o,
                            lhsT=ws_sb[:, 2, ci, co * 128 : (co + 1) * 128],
                            rhs=f_sb[:, ci, psl],
                            start=(ci == 0),
                            stop=(ci == CT - 1),
                        )
                    dve(
                        nc.vector.scalar_tensor_tensor,
                        out=o_ring[:, co, pg, :],
                        in0=ps_o,
                        scalar=bb[:, 2, co : co + 1],
                        in1=xq_sb[:, co, psl],
                        op0=add,
                        op1=add,
                    )
                deferred[0] = None

            for pg in range(PG):
                psl = slice(pg * 512, (pg + 1) * 512)
                ps_f = [
                    psf.tile([128, 512], F32, tag=f"f{ci}", name=f"ps_f{ci}")
                    for ci in range(CT)
                ]
                # software pipeline: PV runs one exp-pair behind scores so
                # the PE streams scores(k+1) while ACT computes exp(k)
                for qp in range(QT // 2 + 1):
                    if qp < QT // 2:
                        ps_s = pss.tile([128, 2, 512], F32, tag="s", bufs=2)
                        for sub in range(2):
                            qt = qp * 2 + sub
                            for ci in range(CT):
                                mm(
                                    ps_s[:, sub],
                                    lhsT=ph_sb[:, ci, qt * 128 : (qt + 1) * 128],
                                    rhs=th_sb[:, ci, psl],
                                    start=(ci == 0),
                                    stop=(ci == CT - 1),
                                )
                        act(
                            e_ring[:, qp % 4], ps_s,
                            mybir.ActivationFunctionType.Exp,
                            bias=zbias, scale=scale,
                        )
                    if qp == 1 and deferred[0] is not None:
                        finalize_bc(pg - 1)
                    if qp == 2 and deferred[0] is not None:
                        finalize_conv(pg - 1)
                    if qp >= 1:
                        qpp = qp - 1
                        e_p = e_ring[:, qpp % 4]
                        # softmax denominator partial sums on DVE
                        if qpp in (0, 8):
                            dst_acc = acc if qpp == 0 else acc2
                            dve(nc.vector.tensor_add,
                                dst_acc, e_p[:, 0], e_p[:, 1])
                        else:
                            dst_acc = acc if qpp < 8 else acc2
                            dve(nc.vector.tensor_add, tsum, e_p[:, 0], e_p[:, 1])
                            dve(nc.vector.tensor_add, dst_acc, dst_acc, tsum)
                        if qpp == 8:
                            # first-half denominators reduce early (overlaps)
                            sum_tiles[0] = pso.tile(
                                [1, 512], F32, tag="misc", name="ps_sum"
                            )
                            mm(sum_tiles[0], lhsT=ones_r, rhs=acc,
                               start=True, stop=False)
                        for sub in range(2):
                            qt = qpp * 2 + sub
                            e_t = e_p[:, sub]
                            first, last_q = qt == 0, qt == QT - 1
                            for ci in range(CT):
                                mm(
                                    ps_f[ci],
                                    lhsT=xt_sb[:, qt, ci * 128 : (ci + 1) * 128],
                                    rhs=e_t,
                                    start=first,
                                    stop=last_q,
                                )

                # second-half reduce into the same PSUM accumulator, then
                # 1/sum via ln+exp (same ACT table set as the softmax exp)
                ps_sum = sum_tiles[0]
                mm(ps_sum, lhsT=ones_r, rhs=acc2, start=False, stop=True)
                act(
                    lns[:, pg], ps_sum, mybir.ActivationFunctionType.Ln,
                    bias=zbias[0:1],
                )
                act(
                    rc_ring[:, pg, :], lns[:, pg],
                    mybir.ActivationFunctionType.Exp,
                    bias=zbias[0:1], scale=-1.0,
                )
                deferred[0] = (pg, ps_f)

            finalize_bc(PG - 1)
            finalize_conv(PG - 1)

            # ---- output stores: one contiguous DMA per channel tile ----
            for co in range(CT):
                nc.sync.dma_start(
                    out=out[co * 128 : (co + 1) * 128, :], in_=o_ring[:, co]
                )
    nc.compile()
    return nc


_PROGRAM_CACHE = {}


def _get_program(mm_dt=mybir.dt.float16):
    key = str(mm_dt)
    if key not in _PROGRAM_CACHE:
        _PROGRAM_CACHE[key] = build_program(P_CORE, HW, C, mm_dt)
    return _PROGRAM_CACHE[key]


def make_in_maps(x, theta_w, theta_b, phi_w, phi_b, conv1_w, conv1_b,
                 mm_np=np.float16):
    """Host-side sharding / layout prep (pure data movement, no math)."""
    wcat = np.ascontiguousarray(
        np.stack(
            [
                np.asarray(theta_w, np.float32).T,
                np.asarray(phi_w, np.float32).T,
                np.asarray(conv1_w, np.float32).T,
            ]
        ).astype(mm_np)
    )
    bcat = np.ascontiguousarray(
        np.stack(
            [
                np.asarray(theta_b, np.float32),
                np.asarray(phi_b, np.float32),
                np.asarray(conv1_b, np.float32),
            ]
        )
    )
    xf = np.asarray(x, np.float32).reshape(N, C, HW)
    in_maps = []
    for core in range(NCORES):
        n, half = divmod(core, CORES_PER_N)
        off = half * P_CORE
        xk_i = np.ascontiguousarray(np.roll(xf[n], -off, axis=1))
        in_maps.append(
            {
                "xk": xk_i.astype(mm_np),
                "xq": np.ascontiguousarray(xk_i[:, :P_CORE]),
                "xt": np.ascontiguousarray(xk_i.T).astype(mm_np),
                "wcat": wcat,
                "bcat": bcat,
            }
        )
    return in_maps


def assemble_output(results):
    y = np.empty((N, C, HW), np.float32)
    for core in range(NCORES):
        n, half = divmod(core, CORES_PER_N)
        off = half * P_CORE
        y[n][:, off : off + P_CORE] = results[core]["out"]
    return y.reshape(N, C, H, W)


def kernel(x, theta_w, theta_b, phi_w, phi_b, conv1_w, conv1_b,
           mm_dt=None, **run_kwargs):
    if mm_dt is None:
        mm_dt = mybir.dt.float16
    nc = _get_program(mm_dt)
    in_maps = make_in_maps(
        x, theta_w, theta_b, phi_w, phi_b, conv1_w, conv1_b,
        mm_np=mybir.dt.np(mm_dt),
    )
    res = run_bass_kernel_spmd(nc, in_maps, list(range(NCORES)), **run_kwargs)
    out = assemble_output(res.results)
    kernel.last_results = res
    return out



# revision 9
# speedup vs baseline: 1.4205x; 1.4205x over previous
"""Trainium2 Bass kernel for nn_DenoisingModule (non-local attention block).

Reference computation (per batch element n, with C=256 channels, HW=4096):
    theta = W_t x + b_t            # queries  [C, HW]
    phi   = W_p x + b_p            # keys     [C, HW]
    g     = x                      # values   [C, HW]
    S     = theta^T phi / sqrt(C)  # [HW, HW]
    A     = softmax(S, axis=keys)
    f     = g A^T                  # [C, HW]
    out   = x + W_c f + b_c

Sharding: 8 cores; each of the N=4 batch elements is split across 2 cores
by query position (2048 queries per core). Every core holds the full key
set for its batch element, so no collectives are needed.

Per-core device program (SPMD, identical on all cores, data differs):
  - scores are computed TRANSPOSED (S^T[q, p] = phi^T theta) so that the
    softmax key-reduction lands on the PSUM partition axis, which lets the
    exp output E^T[q, p] feed the PV matmul directly (no PE transposes).
  - the attention matmuls (scores and PV) run in fp8e4 DoubleRow mode:
    the PE virtualizes to 128x256, doing the full 256-deep contraction in
    one instruction at 2 MACs/cell/cycle. theta/phi are written to fp8 by
    the projection eviction; exp outputs are fp8 with a constant -2 bias
    folded into the activation (softmax is shift-invariant) to keep
    exp values inside fp8e4 range.
  - softmax denominators accumulate on the PE as ones-row DoubleRow
    matmuls into a [1, 512] PSUM slot (one per exp pair, accumulating
    across the whole query group); 1/Sum comes from the DVE
    reciprocal_approx_fast custom op, so the Scalar engine runs ONLY exp
    and never swaps activation tables.
  - residual + bias are fused into the final PSUM eviction; the residual
    is read from the fp16 xk tile (no separate fp32 xq load).
  - the attention loop is software-pipelined (PV trails scores/exp by one
    pair; per-group normalize/conv work is deferred into the next group).

Toolchain constraint that shapes this file: every TPB engine instruction
(and every DMA) may carry at most ONE semaphore wait, so cross-engine
fan-in is funneled through per-engine collector chains, persistent ring
tiles replace rotating tile pools, and loads/stores are merged so each
DMA is the first instruction on its hardware queue.

The host wrapper rolls x columns per-core so queries are always columns
[0, P) of the local key matrix (keeps the program identical across cores),
and pre-transposes x (and the weight matrices) since the PV matmul needs
x^T as the stationary operand.
"""

import numpy as np

import concourse.bass as bass
import concourse.mybir as mybir
from concourse import bacc
from concourse.bass_utils import run_bass_kernel_spmd
from concourse.tile import TileContext, add_dep_helper

N, C, H, W = 4, 256, 64, 64
HW = H * W
NCORES = 8
CORES_PER_N = NCORES // N
P_CORE = HW // CORES_PER_N  # queries per core

F32 = mybir.dt.float32
F32R = mybir.dt.float32r
FP8 = mybir.dt.float8e4
DR = mybir.MatmulPerfMode.DoubleRow

EBIAS = -4.0  # constant shift inside exp (softmax-invariant); keeps
              # exp values in fp8e4 range (<= 240): max observed
              # score*scale is ~7.5 -> e^3.5 = 33, margin up to score 9.4.
              # Keys with score < -2.2 flush to zero weight (< 1e-3 of the
              # softmax mass at this score distribution).


def build_program(P, Q, Cc=C, mm_dt=mybir.dt.float16):
    """Build the per-core Bass program.

    P: queries handled by this core (first P columns of xk)
    Q: total key positions
    mm_dt: dtype for the projection/out-conv matmuls (fp16); the
      attention matmuls are always fp8e4 DoubleRow.
    """
    assert P % 512 == 0 and Q % 512 == 0 and Cc == 256
    CT = Cc // 128
    QT = Q // 128
    PG = P // 512
    QG = Q // 512
    NP = QT // 2  # exp pairs per query group
    scale = float(Cc) ** -0.5

    nc = bacc.Bacc("TRN2", target_bir_lowering=False)
    xk = nc.declare_dram_parameter("xk", [Cc, Q], mm_dt, isOutput=False)[:]
    xt = nc.declare_dram_parameter("xt", [Q, Cc], FP8, isOutput=False)[:]
    wcat = nc.declare_dram_parameter("wcat", [3, Cc, Cc], mm_dt, isOutput=False)[:]
    bcat = nc.declare_dram_parameter("bcat", [3, Cc], F32, isOutput=False)[:]
    out = nc.declare_dram_parameter("out", [Cc, P], F32, isOutput=True)[:]

    add = mybir.AluOpType.add

    with TileContext(nc) as tc:
        with (
            tc.tile_pool(name="const", bufs=1) as const,
            tc.tile_pool(name="big", bufs=1) as big,
            tc.tile_pool(name="pss", bufs=1, space="PSUM") as pss,
            tc.tile_pool(name="psf", bufs=1, space="PSUM") as psf,
            tc.tile_pool(name="pso", bufs=1, space="PSUM") as pso,
        ):
            QH = Q // 2

            # ---- input loads: 5 DMAs, one per HWDGE proc ----
            ws_sb = const.tile([128, 3, CT, Cc], mm_dt, tag="ws")
            w_load = nc.sync.dma_start(
                out=ws_sb, in_=wcat.rearrange("w (a p) o -> p w a o", p=128)
            )
            xk_sb = big.tile([128, CT, Q], mm_dt, tag="xk")
            xk_loads = [
                nc.sync.dma_start(
                    out=xk_sb[:, :, h * QH : (h + 1) * QH],
                    in_=xk[:, h * QH : (h + 1) * QH].rearrange(
                        "(a p) q -> p a q", p=128
                    ),
                )
                for h in range(2)
            ]
            bb = const.tile([128, 3, CT], F32, tag="bb")
            b_load = nc.sync.dma_start(
                out=bb, in_=bcat.rearrange("w (a p) -> p w a", p=128)
            )
            xt_sb = big.tile([128, QT, Cc], FP8, tag="xt")
            xt_load = nc.sync.dma_start(
                out=xt_sb, in_=xt.rearrange("(a p) c -> p a c", p=128)
            )

            # persistent tiles (deliberately NOT pool-rotated: pool-slot
            # releases fan in multiple procs; rings keep wait fan-in low)
            th_sb = big.tile([128, CT, P], FP8, tag="th")
            ph_sb = big.tile([128, CT, Q], FP8, tag="ph")
            f_sb = big.tile([128, CT, P], mm_dt, tag="f")
            e_ring = big.tile([128, 4, 2, 512], FP8, tag="ering")
            rc_ring = const.tile([1, PG, 512], F32R, tag="rcring")
            bc_ring = big.tile([128, PG, 512], F32, tag="bcring")
            o_ring = big.tile([128, CT, PG, 512], F32, tag="oring")

            # ---- engine program-order chains + wait collectors ----
            last = {}

            def chain(eng, inst):
                # ordering edges disabled: Bacc legalizes multi-waits, so the
                # Tile scheduler is free to interleave within each engine
                last[eng] = inst.ins
                return inst

            ones_f = const.tile([128, 1], F32, tag="ones_f")
            chain("v", nc.vector.memset(ones_f, 1.0))
            ones_col = const.tile([1, 128], F32R, tag="ones_col")
            chain("v", nc.vector.tensor_copy(
                ones_col, ones_f[0:1, 0:1].to_broadcast([1, 128])))
            ones8 = const.tile([128, 2, 16], FP8, tag="ones8")
            chain("v", nc.vector.memset(ones8, 1.0))
            zbias = const.tile([128, 1], F32, tag="zbias")
            chain("v", nc.vector.memset(zbias, 0.0))
            ebias = const.tile([128, 1], F32, tag="ebias")
            eb_inst = chain("v", nc.vector.memset(ebias, EBIAS))

            scr_act = const.tile([1, 1], F32, tag="scr_act")
            acol = nc.scalar.activation(
                scr_act, ebias[0:1, :], mybir.ActivationFunctionType.Copy
            )
            add_dep_helper(acol.ins, eb_inst.ins, True, "act bias barrier")
            last["a"] = acol.ins

            for k, ld in enumerate([b_load]):
                scr_k = const.tile([1, 1], F32, tag=f"scr{k}", name=f"scr{k}")
                dcol = nc.vector.memset(scr_k, 0.0)
                add_dep_helper(dcol.ins, ld.ins, True, "dve input barrier")
                chain("v", dcol)

            ps_col = pso.tile([1, 1], F32, tag="misc", name="ps_col")
            probe = bb[0:1, 0, 0:1]

            def pe_barrier(ld):
                col = nc.tensor.matmul(ps_col, lhsT=probe, rhs=probe)
                add_dep_helper(col.ins, ld.ins, True, "pe input barrier")
                chain("p", col)

            pe_barrier(w_load)
            pe_barrier(xk_loads[0])

            def mm(*args, **kwargs):
                return chain("p", nc.tensor.matmul(*args, **kwargs))

            def dve(fn, *args, **kwargs):
                return chain("v", fn(*args, **kwargs))

            def act(*args, **kwargs):
                return chain("a", nc.scalar.activation(*args, **kwargs))

            # ---- projections (paired groups share the "s" PSUM slot) ----
            def project(w_idx, dst, ngroups, bias_col, g0=0):
                for co in range(CT):
                    for g in range(g0, ngroups):
                        ps_pj = psf.tile(
                            [128, 512], F32, tag=f"f{g % 2}", name="ps_pj"
                        )
                        for ci in range(CT):
                            mm(
                                ps_pj,
                                lhsT=ws_sb[:, w_idx, ci, co * 128 : (co + 1) * 128],
                                rhs=xk_sb[:, ci, g * 512 : (g + 1) * 512],
                                start=(ci == 0),
                                stop=(ci == CT - 1),
                            )
                        act(
                            dst[:, co, g * 512 : (g + 1) * 512],
                            ps_pj,
                            mybir.ActivationFunctionType.Identity,
                            bias=bb[:, bias_col, co : co + 1],
                        )

            project(0, th_sb, PG, 0)
            project(1, ph_sb, QG // 2, 1)
            pe_barrier(xk_loads[1])
            project(1, ph_sb, QG, 1, g0=QG // 2)
            pe_barrier(xt_load)

            # ---- attention; per-group finalization is deferred into the
            # next group so the normalize chain (PE ones-reduce -> DVE
            # reciprocal -> PE broadcast -> DVE muls) overlaps PE work
            deferred = [None]

            def finalize_bc(pg):
                psl = slice(pg * 512, (pg + 1) * 512)
                ps_fs = deferred[0][1]
                # shares the "o" bank with the out-conv PSUM: the bc write
                # and its copy-out strictly precede the conv matmuls, so the
                # pool rotation sequences them without a live-range overlap
                # (the "misc" bank stays dedicated to ps_sum, whose
                # accumulation group spans most of the query-group loop).
                ps_bc = pso.tile([128, 512], F32, tag="o", name="ps_bc")
                mm(ps_bc, lhsT=ones_col, rhs=rc_ring[:, pg, :])
                bc_sb = bc_ring[:, pg, :]
                dve(nc.vector.tensor_copy, bc_sb, ps_bc)
                for ci in range(CT):
                    dve(
                        nc.vector.tensor_mul, f_sb[:, ci, psl], ps_fs[ci], bc_sb
                    )

            def finalize_conv(pg):
                psl = slice(pg * 512, (pg + 1) * 512)
                for co in range(CT):
                    ps_o = pso.tile([128, 512], F32, tag="o")
                    for ci in range(CT):
                        mm(
                            ps_o,
                            lhsT=ws_sb[:, 2, ci, co * 128 : (co + 1) * 128],
                            rhs=f_sb[:, ci, psl],
                            start=(ci == 0),
                            stop=(ci == CT - 1),
                        )
                    dve(
                        nc.vector.scalar_tensor_tensor,
                        out=o_ring[:, co, pg, :],
                        in0=ps_o,
                        scalar=bb[:, 2, co : co + 1],
                        in1=xk_sb[:, co, psl],
                        op0=add,
                        op1=add,
                    )
                deferred[0] = None

            for pg in range(PG):
                psl = slice(pg * 512, (pg + 1) * 512)
                ps_f = [
                    psf.tile([128, 512], F32, tag=f"f{ci}", name=f"ps_f{ci}")
                    for ci in range(CT)
                ]
                ps_sum = pso.tile([1, 512], F32, tag="misc", name="ps_sum")

                def denom(qpp, e_p):
                    mm(
                        ps_sum,
                        lhsT=ones8[:, :, 0:1],
                        rhs=e_p[:, 0:2, :],
                        perf_mode=DR,
                        start=(qpp == 0),
                        stop=(qpp == NP - 1),
                    )

                # software pipeline: PV runs one exp-pair behind scores so
                # the PE streams scores(k+1) while ACT computes exp(k)
                for qp in range(NP + 1):
                    if qp < NP:
                        ps_s = pss.tile([128, 2, 512], F32, tag="s", bufs=2)
                        for sub in range(2):
                            qt = qp * 2 + sub
                            mm(
                                ps_s[:, sub],
                                lhsT=ph_sb[:, 0:2, qt * 128 : (qt + 1) * 128],
                                rhs=th_sb[:, 0:2, psl],
                                perf_mode=DR,
                                start=True,
                                stop=True,
                            )
                        act(
                            e_ring[:, qp % 4], ps_s,
                            mybir.ActivationFunctionType.Exp,
                            bias=ebias, scale=scale,
                        )
                    if qp == 1 and deferred[0] is not None:
                        finalize_bc(pg - 1)
                    if qp == 2 and deferred[0] is not None:
                        finalize_conv(pg - 1)
                    if qp >= 1:
                        qpp = qp - 1
                        e_p = e_ring[:, qpp % 4]
                        # softmax denominators: ones-row DoubleRow matmuls
                        # accumulating into ps_sum.  The qpp==0 pair is
                        # deferred to qp==2 so the misc PSUM bank has been
                        # released by the previous group's broadcast.
                        if qpp == 1:
                            denom(0, e_ring[:, 0])
                            denom(1, e_p)
                        elif qpp >= 2:
                            denom(qpp, e_p)
                        for ci in range(CT):
                            mm(
                                ps_f[ci],
                                lhsT=xt_sb[
                                    :, 2 * qpp : 2 * qpp + 2,
                                    ci * 128 : (ci + 1) * 128,
                                ],
                                rhs=e_p[:, 0:2, :],
                                perf_mode=DR,
                                start=(qpp == 0),
                                stop=(qpp == NP - 1),
                            )

                # 1/sum on DVE (custom op; ~18 correct bits, no ACT table).
                # Emitted via _custom_dve so the output can be declared
                # float32r (same bits as fp32) for the broadcast matmul.
                from concourse.dve_ops import (
                    RECIP_APPROX_FAST_CONSTS,
                    RECIPROCAL_APPROX_FAST,
                )

                rc = RECIP_APPROX_FAST_CONSTS
                dve(
                    nc.vector._custom_dve,
                    RECIPROCAL_APPROX_FAST,
                    out=rc_ring[:, pg, :],
                    in0=ps_sum,
                    s0=rc["s0"],
                    s1=rc["s1"],
                    imm2=rc["imm2"],
                )
                deferred[0] = (pg, ps_f)

            finalize_bc(PG - 1)
            finalize_conv(PG - 1)

            # ---- output stores: one contiguous DMA per channel tile ----
            for co in range(CT):
                nc.sync.dma_start(
                    out=out[co * 128 : (co + 1) * 128, :], in_=o_ring[:, co]
                )
    nc.compile()
    return nc


_PROGRAM_CACHE = {}


def _get_program(mm_dt=mybir.dt.float16):
    key = str(mm_dt)
    if key not in _PROGRAM_CACHE:
        _PROGRAM_CACHE[key] = build_program(P_CORE, HW, C, mm_dt)
    return _PROGRAM_CACHE[key]


def make_in_maps(x, theta_w, theta_b, phi_w, phi_b, conv1_w, conv1_b,
                 mm_np=np.float16):
    """Host-side sharding / layout prep (pure data movement, no math)."""
    fp8_np = mybir.dt.np(FP8)
    wcat = np.ascontiguousarray(
        np.stack(
            [
                np.asarray(theta_w, np.float32).T,
                np.asarray(phi_w, np.float32).T,
                np.asarray(conv1_w, np.float32).T,
            ]
        ).astype(mm_np)
    )
    bcat = np.ascontiguousarray(
        np.stack(
            [
                np.asarray(theta_b, np.float32),
                np.asarray(phi_b, np.float32),
                np.asarray(conv1_b, np.float32),
            ]
        )
    )
    xf = np.asarray(x, np.float32).reshape(N, C, HW)
    in_maps = []
    for core in range(NCORES):
        n, half = divmod(core, CORES_PER_N)
        off = half * P_CORE
        xk_i = np.ascontiguousarray(np.roll(xf[n], -off, axis=1))
        in_maps.append(
            {
                "xk": xk_i.astype(mm_np),
                "xt": np.ascontiguousarray(xk_i.T).astype(fp8_np),
                "wcat": wcat,
                "bcat": bcat,
            }
        )
    return in_maps


def assemble_output(results):
    y = np.empty((N, C, HW), np.float32)
    for core in range(NCORES):
        n, half = divmod(core, CORES_PER_N)
        off = half * P_CORE
        y[n][:, off : off + P_CORE] = results[core]["out"]
    return y.reshape(N, C, H, W)


def kernel(x, theta_w, theta_b, phi_w, phi_b, conv1_w, conv1_b,
           mm_dt=None, **run_kwargs):
    if mm_dt is None:
        mm_dt = mybir.dt.float16
    nc = _get_program(mm_dt)
    in_maps = make_in_maps(
        x, theta_w, theta_b, phi_w, phi_b, conv1_w, conv1_b,
        mm_np=mybir.dt.np(mm_dt),
    )
    res = run_bass_kernel_spmd(nc, in_maps, list(range(NCORES)), **run_kwargs)
    out = assemble_output(res.results)
    kernel.last_results = res
    return out
